# revision 34
# baseline (speedup 1.0000x reference)
"""ConditionalRandomField loss kernel for Trainium2 (8 NeuronCores).

Math (per sequence b):
    loss[b] = log_score(gold path) - log_partition

log_partition via a CHUNK-PARALLEL scan in exp space: each sequence's
1024 steps are split into C=64 chunks of K=16 scanned simultaneously as
independent chains (8 seqs x 64 chunks = 512 matmul columns per step in
2 pipelined groups), so the per-step PE->PSUM->DVE->PE round trip is
amortized over 512 chains instead of serializing 512+ tiny steps.
Chunks c>=1 start from ones and run W=3 warmup steps; products of
random positive matrices contract to rank-1 in a few steps, so after W
steps the chain direction equals the true forward state's direction.
Per-chunk scales are stitched with column-sum dot products:

  logZ = ln(1.psi_0) + sum_{c=1}^{C-2} ln(1.psi_c) - sum_{c=1}^{C-1} ln(1.h_c)
       + ln(stop.psi_{C-1}) + 1024*S

where h_c = state at warmup end (slab W), psi_c = state at chunk end
(slab G), E = exp(transitions) in fp8 (PE weights), g_t = exp(emit_t-S)
with S = 6.5 keeping magnitudes flat.  Validated in numpy: abs logZ
error ~0.6 (fp8 dominated) vs tolerance ~130.

The drain is split PSUM->SBUF copy on ACT + 2x-mode bf16 multiply on
DVE (gbuf is slab-major so every DVE operand is packed bf16).  Warmup
g-slabs duplicate the previous chunk's head, so they are built with
cheap 4x-mode copies instead of ACT exp.

Numerator: host computes per-seq transition-pair COUNT matrices from
tags (integer bookkeeping only; float math stays on device):
score_tr[b] = <Count_b, transitions> with start/stop folded in as
one-hot ext columns.  Gold emissions via on-device one-hot masks (iota
compare, 4x mode) and 2x-mode products, column-summed on the PE with
ones-matmuls into per-seq PSUM regions; both gathers share one PSUM
accumulation region per sequence.

Sharding: data-parallel over batch; core c owns sequences [8c, 8c+8).

NOTE: mask is all-ones for this problem spec (fill: ones); the kernel
assumes it (the reference's masked branches are identities then).
"""

import numpy as np
from contextlib import ExitStack

import concourse.bass as bass
import concourse.bacc as bacc
import concourse.tile as tile
from concourse import mybir
from concourse.bass_utils import run_bass_kernel_spmd

F32 = mybir.dt.float32
BF16 = mybir.dt.bfloat16
FP8 = mybir.dt.float8e4

NCORES = 8
B = 64
L = 1024
T = 256
BC = B // NCORES      # sequences per core
JCN = T // 128        # = 2 tag chunks
S = 6.5               # log-shift folded into g = exp(emit - S)

CCH = 64              # chunks per sequence
KK = L // CCH         # chunk length (16)
W = 1                 # warmup steps
G = W + KK            # slabs per chain (18); slab 0 = init-time g only
NCH = BC * CCH        # chains per core (512); chain = c*BC + b
GRP = NCH // 2        # chains per pipeline group (256)
NPIECE = 4            # em DMA pieces (by t-range)
TP = L // NPIECE      # 256 t per piece

CNTC = 2 * T + 4      # count-matrix cols per seq: [i_hi,j] + 4 one-hot ext
AUX_N = T * T + 2 * T # aux: [trans i-major | start | stop]
GSTRIDE = (G + 1) * NCH   # per-jc stride in gbuf


def build_program(debug=False):
    nc = bacc.Bacc()
    em_t = nc.declare_dram_parameter("em", [128 * JCN * BC * L, 1], FP8, isOutput=False)
    em2_t = nc.declare_dram_parameter("em2", [128 * JCN * BC * L, 1], BF16, isOutput=False)
    tags_t = nc.declare_dram_parameter("tags_sc", [BC * L, 1], BF16, isOutput=False)
    cnt_t = nc.declare_dram_parameter("cnt", [128 * BC * CNTC, 1], BF16, isOutput=False)
    aux_t = nc.declare_dram_parameter("aux", [AUX_N, 1], F32, isOutput=False)
    iota_t = nc.declare_dram_parameter("iota", [128, 1], F32, isOutput=False)
    loss_t = nc.declare_dram_parameter("loss", [BC, 1], F32, isOutput=True)
    dbg_t = nc.declare_dram_parameter("dbg", [4096, 1], F32, isOutput=True) if debug else None

    def dram_ap(handle, offset, ap):
        full = handle[:]
        return bass.AP(tensor=full.tensor, offset=offset, ap=ap)

    with tile.TileContext(nc) as tc, ExitStack() as ctx:
        const = ctx.enter_context(tc.tile_pool(name="const", bufs=1))
        stage = ctx.enter_context(tc.tile_pool(name="stage", bufs=2))
        fpA = ctx.enter_context(tc.tile_pool(name="fpA", bufs=3))
        fpB = ctx.enter_context(tc.tile_pool(name="fpB", bufs=3))
        prp = ctx.enter_context(tc.tile_pool(name="prp", bufs=8))
        pp = ctx.enter_context(tc.tile_pool(name="pp", bufs=3, space="PSUM"))
        psp = ctx.enter_context(tc.tile_pool(name="psp", bufs=1, space="PSUM"))

        # ---------------- DMAs --------------------------------------------
        # sync queue: aux + fp8 emissions (scan path, earliest)
        # ACT queue:  tags broadcast + count matrices
        # DVE queue:  bf16 emissions (numerator products)
        iota_sb = const.tile([128, 1], F32, name="iota_sb")
        nc.scalar.dma_start(out=iota_sb, in_=iota_t[:])
        neg_shift = const.tile([128, 1], F32, name="neg_shift")
        nc.vector.memset(neg_shift, -S)

        eraw = [stage.tile([128, T], F32, name=f"eraw{ic}", tag="eraw") for ic in range(JCN)]
        for ic in range(JCN):
            nc.scalar.dma_start(
                out=eraw[ic], in_=dram_ap(aux_t, ic * 128 * T, [[T, 128], [1, T]])
            )
        ssraw = const.tile([128, 2 * JCN], F32, name="ssraw")
        nc.scalar.dma_start(
            out=ssraw[:, 0:JCN], in_=dram_ap(aux_t, T * T, [[1, 128], [128, JCN]])
        )
        nc.scalar.dma_start(
            out=ssraw[:, JCN:2 * JCN],
            in_=dram_ap(aux_t, T * T + T, [[1, 128], [128, JCN]]),
        )

        raw = const.tile([128, JCN, BC, L], FP8, name="raw")
        raw2 = const.tile([128, JCN, BC, L], BF16, name="raw2")
        for p in range(NPIECE):
            dst = bass.AP(
                tensor=raw.tensor,
                offset=raw.offset + p * TP,
                ap=[raw.ap[0], [BC * L, JCN], [L, BC], [1, TP]],
            )
            nc.sync.dma_start(
                out=dst,
                in_=dram_ap(
                    em_t, p * 128 * JCN * BC * TP,
                    [[JCN * BC * TP, 128], [1, JCN * BC * TP]],
                ),
            )
        for p in range(NPIECE):
            dst = bass.AP(
                tensor=raw2.tensor,
                offset=raw2.offset + p * TP,
                ap=[raw2.ap[0], [BC * L, JCN], [L, BC], [1, TP]],
            )
            nc.sync.dma_start(
                out=dst,
                in_=dram_ap(
                    em2_t, p * 128 * JCN * BC * TP,
                    [[JCN * BC * TP, 128], [1, JCN * BC * TP]],
                ),
            )

        tags_bc = const.tile([128, BC * L], BF16, name="tags_bc")
        nc.scalar.dma_start(
            out=tags_bc, in_=dram_ap(tags_t, 0, [[0, 128], [1, BC * L]])
        )
        cnt_sb = const.tile([128, BC, CNTC], BF16, name="cnt_sb")
        nc.scalar.dma_start(
            out=cnt_sb, in_=dram_ap(cnt_t, 0, [[BC * CNTC, 128], [1, BC * CNTC]])
        )

        # ---------------- weights + start/stop ----------------
        e_tiles = []
        trext = const.tile([128, CNTC], BF16, name="trext")
        for ic in range(JCN):
            ebf = const.tile([128, T], FP8, name=f"ebf{ic}")
            nc.scalar.activation(out=ebf, in_=eraw[ic], func=mybir.ActivationFunctionType.Exp)
            e_tiles.append(ebf)
            nc.vector.tensor_copy(out=trext[:, ic * T:(ic + 1) * T], in_=eraw[ic])
        nc.vector.tensor_copy(out=trext[:, 2 * T:2 * T + 4], in_=ssraw)
        sstart = const.tile([128, JCN], F32, name="sstart")
        nc.scalar.activation(
            out=sstart, in_=ssraw[:, 0:JCN], func=mybir.ActivationFunctionType.Exp
        )
        sstop_bf = const.tile([128, JCN], BF16, name="sstop_bf")
        nc.scalar.activation(
            out=sstop_bf, in_=ssraw[:, JCN:2 * JCN], func=mybir.ActivationFunctionType.Exp
        )
        ones_col = const.tile([128, 1], BF16, name="ones_col")
        nc.vector.memset(ones_col, 1.0)

        # ---------------- g = exp(emit - S), chain-major -------------------
        # gbuf [128, jc, chain, slab]; chain = c*BC + b; slab s>=1 applies
        # g(t): chain 0: t = s; chains c>=1: t = c*K - W + s - 1.
        # chain-major keeps the ACT exp writes slab-contiguous (strided ACT
        # output measured 6.3us/call on hw); the fused drain reads g strided
        # which costs nothing extra (PSUM input already blocks DVE 2x mode).
        gbuf = const.tile([128, JCN, NCH, G + 1], BF16, name="gbuf")

        def emit_exp_chain0(jc):
            out_ap = bass.AP(
                tensor=gbuf.tensor,
                offset=gbuf.offset + jc * GSTRIDE,
                ap=[gbuf.ap[0], [G + 1, BC], [1, G + 1]],
            )
            in_ap = bass.AP(
                tensor=raw.tensor,
                offset=raw.offset + jc * BC * L,
                ap=[raw.ap[0], [L, BC], [1, G + 1]],
            )
            nc.scalar.activation(
                out=out_ap, in_=in_ap, func=mybir.ActivationFunctionType.Exp,
                bias=neg_shift[:],
            )

        def emit_exp(jc, c0, ncnk):
            # chunks c0..c0+ncnk: slabs 1..G  <->  t = c*K - W + s - 1
            out_ap = bass.AP(
                tensor=gbuf.tensor,
                offset=gbuf.offset + jc * GSTRIDE + c0 * BC * (G + 1) + 1,
                ap=[gbuf.ap[0], [BC * (G + 1), ncnk], [G + 1, BC], [1, G]],
            )
            in_ap = bass.AP(
                tensor=raw.tensor,
                offset=raw.offset + jc * BC * L + c0 * KK - W,
                ap=[raw.ap[0], [KK, ncnk], [L, BC], [1, G]],
            )
            nc.scalar.activation(
                out=out_ap, in_=in_ap, func=mybir.ActivationFunctionType.Exp,
                bias=neg_shift[:],
            )

        # A group: chunks 0..31 (em pieces 0,1); B group: chunks 32..63
        for jc in range(JCN):
            emit_exp_chain0(jc)
        for jc in range(JCN):
            emit_exp(jc, 1, 15)
            emit_exp(jc, 16, 16)
        for jc in range(JCN):
            emit_exp(jc, 32, 16)
            emit_exp(jc, 48, 16)

        # ---------------- one-hot masks ----------------
        oh_tiles = [const.tile([128, BC * L], BF16, name=f"oh{jc}") for jc in range(JCN)]
        for jc in range(JCN):
            for hh in range(2):
                lo, hi = hh * (BC * L // 2), (hh + 1) * (BC * L // 2)
                nc.vector.tensor_scalar(
                    out=oh_tiles[jc][:, lo:hi],
                    in0=tags_bc[:, lo:hi],
                    scalar1=float(jc * 128),
                    scalar2=iota_sb[:],
                    op0=mybir.AluOpType.subtract,
                    op1=mybir.AluOpType.is_equal,
                )

        # ---------------- numerator side-jobs (interleaved into the scan) --
        # psE [1, 512]: per-seq [b*64, 64] accumulation region; emissions
        # (2 jc x 2 halves) + count products all column-summed into it.
        psE = psp.tile([1, 8 * 64], F32, name="psE", tag="psE")
        psE2 = psp.tile([1, 8 * 64], F32, name="psE2", tag="psE2")
        seq_mm_count = [0] * BC
        SEQ_MM_TOTAL = JCN * 2 * 8 + 9  # 16 emission-chunk mms x2 + 9 count mms

        def emit_prod_tt(job, eng):
            kind = job[0]
            if kind == "emis":
                _, jc, b, hh = job
                lo = hh * (L // 2)
                tg = "prod" if eng is nc.gpsimd else "prodD"
                pr = prp.tile([128, L // 2], BF16, name="prod", tag=tg)
                eng.tensor_tensor(
                    out=pr,
                    in0=bass.AP(
                        tensor=raw2.tensor,
                        offset=raw2.offset + jc * BC * L + b * L + lo,
                        ap=[raw2.ap[0], [1, L // 2]],
                    ),
                    in1=oh_tiles[jc][:, b * L + lo: b * L + lo + L // 2],
                    op=mybir.AluOpType.mult,
                )
            else:
                _, b = job
                pr = prp.tile([128, CNTC], BF16, name="prodT", tag="prodT")
                eng.tensor_tensor(
                    out=pr, in0=cnt_sb[:, b, :], in1=trext, op=mybir.AluOpType.mult
                )
            return pr

        def mm_entries(pr, b, ps):
            n = (pr.shape[-1] + 63) // 64
            return [(pr, k, b, ps) for k in range(n)]

        def emit_mm(e, start, stop):
            pr, k, b, ps = e
            c0 = k * 64
            c1 = min((k + 1) * 64, pr.shape[-1])
            nc.tensor.matmul(
                out=ps[:, b * 64:b * 64 + (c1 - c0)],
                lhsT=ones_col,
                rhs=pr[:, c0:c1],
                start=start,
                stop=stop,
            )

        def emit_prod_job(job, eng, start, stop, ps=None):
            pr = emit_prod_tt(job, eng)
            ents = mm_entries(pr, job[2] if job[0] == "emis" else job[1], ps)
            for i, e in enumerate(ents):
                emit_mm(e, start and i == 0, stop and i == len(ents) - 1)

        # ONE accumulation group per PSUM region (the scheduler reorders
        # groups freely, so start=False continuation groups get erased by a
        # later-scheduled start=True group).  GPSIMD products (emission
        # half-0 + count) accumulate in psE; DVE products (half-1) in psE2.
        gp_mms = []
        for b in range(BC):
            ents = []
            for jc in range(JCN):
                ents += mm_entries(emit_prod_tt(("emis", jc, b, 0), nc.gpsimd), b, psE)
            ents += mm_entries(emit_prod_tt(("cnt", b), nc.gpsimd), b, psE)
            gp_mms.append(ents)
        gp_flat = []
        for ents in gp_mms:
            for i, e in enumerate(ents):
                gp_flat.append((e, i == 0, i == len(ents) - 1))
        hh1_jobs = [("emis", jc, b, 1) for b in range(BC) for jc in range(JCN)]

        # ---------------- scan init ----------------
        fwA = fpA.tile([128, JCN, GRP], BF16, name="fwA", tag="fwA")
        fwB = fpB.tile([128, JCN, GRP], BF16, name="fwB", tag="fwB")
        nc.vector.memset(fwA, 1.0)
        nc.vector.memset(fwB, 1.0)
        for jc in range(JCN):
            nc.vector.tensor_scalar_mul(
                out=fwA[:, jc, 0:BC],
                in0=bass.AP(tensor=gbuf.tensor, offset=gbuf.offset + jc * GSTRIDE,
                            ap=[gbuf.ap[0], [G + 1, BC]]),
                scalar1=sstart[:, jc:jc + 1],
            )

        psH = psp.tile([1, NCH], F32, name="psH", tag="psH")
        psPsi = psp.tile([1, NCH], F32, name="psPsi", tag="psPsi")
        psB = psp.tile([1, 2 * BC], F32, name="psB", tag="psB")

        def g_drain_view(grp_base, s):
            return bass.AP(
                tensor=gbuf.tensor,
                offset=gbuf.offset + grp_base * (G + 1) + s,
                ap=[gbuf.ap[0], [GSTRIDE, JCN], [G + 1, GRP]],
            )

        def g_drain_view_o(grp_base, o, s):
            return bass.AP(
                tensor=gbuf.tensor,
                offset=gbuf.offset + o * GSTRIDE + grp_base * (G + 1) + s,
                ap=[gbuf.ap[0], [G + 1, GRP]],
            )

        def scan_slab(fwA_t, fwB_t, s):
            pfA = pp.tile([128, JCN, GRP], F32, name="pfA", tag="pf")
            pfB = pp.tile([128, JCN, GRP], F32, name="pfB", tag="pf")
            fwA2 = fpA.tile([128, JCN, GRP], BF16, name="fwA", tag="fwA")
            fwB2 = fpB.tile([128, JCN, GRP], BF16, name="fwB", tag="fwB")
            for (pf, fw_t) in ((pfA, fwA_t), (pfB, fwB_t)):
                for o in range(JCN):
                    for ic in range(JCN):
                        nc.tensor.matmul(
                            out=pf[:, o, :],
                            lhsT=e_tiles[ic][:, o * 128:(o + 1) * 128],
                            rhs=fw_t[:, ic, :],
                            start=(ic == 0),
                            stop=(ic == JCN - 1),
                        )
            for (pf, fw2, gb) in ((pfA, fwA2, 0), (pfB, fwB2, GRP)):
                nc.vector.tensor_tensor(
                    out=fw2, in0=pf,
                    in1=bass.AP(tensor=gbuf.tensor,
                                offset=gbuf.offset + gb * (G + 1) + s,
                                ap=[gbuf.ap[0], [GSTRIDE, JCN], [G + 1, GRP]]),
                    op=mybir.AluOpType.mult,
                )
            return fwA2, fwB2

        def extract_colsums(ps_region, fw_tile, lhsT, c0=0, n=GRP):
            for jc in range(JCN):
                nc.tensor.matmul(
                    out=ps_region,
                    lhsT=lhsT,
                    rhs=fw_tile[:, jc, c0:c0 + n],
                    start=(jc == 0),
                    stop=(jc == JCN - 1),
                )

        for s in range(1, G + 1):
            fwA, fwB = scan_slab(fwA, fwB, s)
            if 8 <= s <= 15:
                for j in range(2):
                    jc_, b_ = hh1_jobs[(s - 8) * 2 + j][1], hh1_jobs[(s - 8) * 2 + j][2]
                    emit_prod_job(("emis", jc_, b_, 1), nc.vector,
                                  start=(jc_ == 0), stop=(jc_ == JCN - 1), ps=psE2)
            lo_m = max(0, (s - 3)) * len(gp_flat) // 14
            hi_m = (s - 2) * len(gp_flat) // 14 if s <= 16 else len(gp_flat)
            if 3 <= s <= 16:
                for (e, st_, sp_) in gp_flat[lo_m:hi_m]:
                    emit_mm(e, st_, sp_)
            if s == W:
                extract_colsums(psH[:, 0:GRP], fwA, ones_col)
                extract_colsums(psH[:, GRP:NCH], fwB, ones_col)
            if s == KK - 1:
                extract_colsums(psB[:, 0:BC], fwA, ones_col, 0, BC)
            if s == G:
                extract_colsums(psPsi[:, 0:GRP], fwA, ones_col)
                extract_colsums(psPsi[:, GRP:NCH], fwB, ones_col)
                for jc in range(JCN):
                    nc.tensor.matmul(
                        out=psB[:, BC:2 * BC],
                        lhsT=sstop_bf[:, jc:jc + 1],
                        rhs=fwB[:, jc, GRP - BC:GRP],
                        start=(jc == 0),
                        stop=(jc == JCN - 1),
                    )

        # ---------------- finalization ----------------
        lnH = const.tile([1, NCH], F32, name="lnH")
        nc.scalar.activation(out=lnH, in_=psH, func=mybir.ActivationFunctionType.Ln)
        lnPsi = const.tile([1, NCH], F32, name="lnPsi")
        nc.scalar.activation(out=lnPsi, in_=psPsi, func=mybir.ActivationFunctionType.Ln)
        lnB = const.tile([1, 2 * BC], F32, name="lnB")
        nc.scalar.activation(out=lnB, in_=psB, func=mybir.ActivationFunctionType.Ln)

        SH = const.tile([1, BC], F32, name="SH")
        nc.vector.tensor_reduce(
            out=SH,
            in_=bass.AP(tensor=lnH.tensor, offset=lnH.offset + BC,
                        ap=[lnH.ap[0], [1, BC], [BC, CCH - 1]]),
            axis=mybir.AxisListType.X, op=mybir.AluOpType.add,
        )
        SA = const.tile([1, BC], F32, name="SA")
        nc.vector.tensor_reduce(
            out=SA,
            in_=bass.AP(tensor=lnPsi.tensor, offset=lnPsi.offset + BC,
                        ap=[lnPsi.ap[0], [1, BC], [BC, CCH - 2]]),
            axis=mybir.AxisListType.X, op=mybir.AluOpType.add,
        )
        numer1 = const.tile([1, BC], F32, name="numer1")
        nc.vector.tensor_reduce(
            out=numer1,
            in_=bass.AP(tensor=psE.tensor, offset=psE.offset,
                        ap=[psE.ap[0], [64, BC], [1, 64]]),
            axis=mybir.AxisListType.X, op=mybir.AluOpType.add,
        )
        numer2 = const.tile([1, BC], F32, name="numer2")
        nc.vector.tensor_reduce(
            out=numer2,
            in_=bass.AP(tensor=psE2.tensor, offset=psE2.offset,
                        ap=[psE2.ap[0], [64, BC], [1, 64]]),
            axis=mybir.AxisListType.X, op=mybir.AluOpType.add,
        )
        numer = const.tile([1, BC], F32, name="numer")
        nc.vector.tensor_tensor(out=numer, in0=numer1, in1=numer2, op=mybir.AluOpType.add)
        # loss = (numer - L*S) + (SH - SA - lnP0 - lnStop)
        u1 = const.tile([1, BC], F32, name="u1")
        nc.vector.tensor_tensor(out=u1, in0=SH, in1=SA, op=mybir.AluOpType.subtract)
        u2 = const.tile([1, BC], F32, name="u2")
        nc.vector.tensor_tensor(out=u2, in0=u1, in1=lnB[:, 0:BC], op=mybir.AluOpType.subtract)
        u3 = const.tile([1, BC], F32, name="u3")
        nc.vector.tensor_tensor(out=u3, in0=u2, in1=lnB[:, BC:2 * BC], op=mybir.AluOpType.subtract)
        loss_sb = const.tile([1, BC], F32, name="loss_sb")
        nc.vector.scalar_tensor_tensor(
            out=loss_sb,
            in0=numer,
            scalar=float(L * S),
            in1=u3,
            op0=mybir.AluOpType.subtract,
            op1=mybir.AluOpType.add,
        )
        nc.sync.dma_start(out=dram_ap(loss_t, 0, [[1, 1], [1, BC]]), in_=loss_sb)
        if debug:
            psOH = psp.tile([1, 512], F32, name="psOH", tag="psOH")
            for jc in range(JCN):
                nc.tensor.matmul(out=psOH, lhsT=ones_col, rhs=oh_tiles[jc][:, 0:512],
                                 start=(jc == 0), stop=(jc == JCN - 1))
            psOH_sb = const.tile([1, 512], F32, name="psOH_sb")
            nc.vector.tensor_copy(out=psOH_sb, in_=psOH)
            nc.sync.dma_start(out=dram_ap(dbg_t, 2 * NCH + 3 * BC + 512, [[1, 1], [1, 512]]), in_=psOH_sb)
            r2d = const.tile([1, 512], F32, name="r2d")
            nc.vector.tensor_copy(
                out=r2d,
                in_=bass.AP(tensor=raw2.tensor, offset=raw2.offset,
                            ap=[[raw2.ap[0][0], 1], [1, 512]]),
            )
            nc.sync.dma_start(out=dram_ap(dbg_t, 2 * NCH + 3 * BC + 1024, [[1, 1], [1, 512]]), in_=r2d)
            psE_sb = const.tile([1, 8 * 64], F32, name="psE_sb")
            nc.vector.tensor_copy(out=psE_sb, in_=psE)
            nc.sync.dma_start(out=dram_ap(dbg_t, 2 * NCH + 3 * BC, [[1, 1], [1, 8 * 64]]), in_=psE_sb)
            nc.sync.dma_start(out=dram_ap(dbg_t, 0, [[1, 1], [1, NCH]]), in_=lnH)
            nc.sync.dma_start(out=dram_ap(dbg_t, NCH, [[1, 1], [1, NCH]]), in_=lnPsi)
            nc.sync.dma_start(out=dram_ap(dbg_t, 2 * NCH, [[1, 1], [1, 2 * BC]]), in_=lnB)
            nc.sync.dma_start(out=dram_ap(dbg_t, 2 * NCH + 2 * BC, [[1, 1], [1, BC]]), in_=numer)

    nc.finalize()
    return nc


def host_inputs(inputs, tags, length=L):
    """Per-core relaid inputs (host-side sharding / layout / int bookkeeping)."""
    inputs = np.asarray(inputs, dtype=np.float32)
    tags = np.asarray(tags).astype(np.int64)
    fp8 = mybir.dt.np(FP8)
    bf16 = mybir.dt.np(BF16)

    in_maps = []
    for cc in range(NCORES):
        bsl = slice(cc * BC, (cc + 1) * BC)
        x = inputs[bsl].reshape(BC, length, JCN, 128)
        # em[p, part, jc, seq, t'] with t = p*TP + t'
        em = np.ascontiguousarray(
            x.reshape(BC, NPIECE, TP, JCN, 128).transpose(1, 4, 3, 0, 2)
        )
        tg = tags[bsl].astype(bf16).reshape(-1)
        cnt = np.zeros((128, BC, CNTC), np.float32)
        tgs = tags[bsl]
        for b in range(BC):
            c2 = np.zeros((128, 2, T), np.float32)
            np.add.at(c2, (tgs[b, :-1] % 128, tgs[b, :-1] // 128, tgs[b, 1:]), 1.0)
            cnt[:, b, :2 * T] = c2.reshape(128, 2 * T)
            cnt[tgs[b, 0] % 128, b, 2 * T + tgs[b, 0] // 128] = 1.0
            cnt[tgs[b, -1] % 128, b, 2 * T + 2 + tgs[b, -1] // 128] = 1.0
        in_maps.append(dict(
            em=em.astype(fp8).reshape(-1, 1),
            em2=em.astype(bf16).reshape(-1, 1),
            tags_sc=tg.reshape(-1, 1),
            cnt=cnt.astype(bf16).reshape(-1, 1),
        ))
    return in_maps


def host_shared(transitions, start_transitions, stop_transitions):
    aux = np.zeros((AUX_N, 1), np.float32)
    aux[:T * T, 0] = np.asarray(transitions, np.float32).reshape(-1)  # i-major
    aux[T * T:T * T + T, 0] = np.asarray(start_transitions, np.float32)
    aux[T * T + T:, 0] = np.asarray(stop_transitions, np.float32)
    iota = np.arange(128, dtype=np.float32).reshape(128, 1)
    return dict(aux=aux, iota=iota)


def kernel(inputs, tags, mask, transitions, start_transitions, stop_transitions):
    del mask  # all-ones per the problem spec
    in_maps = host_inputs(inputs, tags)
    shared = host_shared(transitions, start_transitions, stop_transitions)
    for m in in_maps:
        m.update(shared)

    nc = build_program()
    res = run_bass_kernel_spmd(nc, in_maps, core_ids=list(range(NCORES)))
    out = np.concatenate([r["loss"].reshape(BC) for r in res.results])
    return out.astype(np.float32)


if __name__ == "__main__":
    rng = np.random.default_rng(0)
    inputs = rng.standard_normal((B, L, T), dtype=np.float32)
    tags = rng.integers(0, T, size=(B, L))
    trans = rng.standard_normal((T, T)).astype(np.float32)
    start = rng.standard_normal(T).astype(np.float32)
    stop = rng.standard_normal(T).astype(np.float32)
    out = kernel(inputs, tags, np.ones((B, L), bool), trans, start, stop)
    print(out)


# revision 35
# speedup vs baseline: 1.1355x; 1.1355x over previous
"""ConditionalRandomField loss kernel for Trainium2 (8 NeuronCores).

Math (per sequence b):
    loss[b] = log_score(gold path) - log_partition

log_partition via a CHUNK-PARALLEL scan in exp space: each sequence's
1024 steps are split into C=64 chunks of K=16 scanned simultaneously as
independent chains (8 seqs x 64 chunks = 512 matmul columns per step in
2 pipelined groups), so the per-step PE->PSUM->DVE->PE round trip is
amortized over 512 chains instead of serializing 512+ tiny steps.
Chunks c>=1 start from ones and run W=3 warmup steps; products of
random positive matrices contract to rank-1 in a few steps, so after W
steps the chain direction equals the true forward state's direction.
Per-chunk scales are stitched with column-sum dot products:

  logZ = ln(1.psi_0) + sum_{c=1}^{C-2} ln(1.psi_c) - sum_{c=1}^{C-1} ln(1.h_c)
       + ln(stop.psi_{C-1}) + 1024*S

where h_c = state at warmup end (slab W), psi_c = state at chunk end
(slab G), E = exp(transitions) in fp8 (PE weights), g_t = exp(emit_t-S)
with S = 6.5 keeping magnitudes flat.  Validated in numpy: abs logZ
error ~0.6 (fp8 dominated) vs tolerance ~130.

The drain is split PSUM->SBUF copy on ACT + 2x-mode bf16 multiply on
DVE (gbuf is slab-major so every DVE operand is packed bf16).  Warmup
g-slabs duplicate the previous chunk's head, so they are built with
cheap 4x-mode copies instead of ACT exp.

Numerator: host computes per-seq transition-pair COUNT matrices from
tags (integer bookkeeping only; float math stays on device):
score_tr[b] = <Count_b, transitions> with start/stop folded in as
one-hot ext columns.  Gold emissions via on-device one-hot masks (iota
compare, 4x mode) and 2x-mode products, column-summed on the PE with
ones-matmuls into per-seq PSUM regions; both gathers share one PSUM
accumulation region per sequence.

Sharding: data-parallel over batch; core c owns sequences [8c, 8c+8).

NOTE: mask is all-ones for this problem spec (fill: ones); the kernel
assumes it (the reference's masked branches are identities then).
"""

import numpy as np
from contextlib import ExitStack

import concourse.bass as bass
import concourse.bacc as bacc
import concourse.tile as tile
from concourse import mybir
from concourse.bass_utils import run_bass_kernel_spmd

F32 = mybir.dt.float32
BF16 = mybir.dt.bfloat16
FP8 = mybir.dt.float8e4

NCORES = 8
B = 64
L = 1024
T = 256
BC = B // NCORES      # sequences per core
JCN = T // 128        # = 2 tag chunks
S = 6.5               # log-shift folded into g = exp(emit - S)

CCH = 64              # chunks per sequence
KK = L // CCH         # chunk length (16)
W = 1                 # warmup steps
G = W + KK            # slabs per chain (18); slab 0 = init-time g only
NCH = BC * CCH        # chains per core (512); chain = c*BC + b
GRP = NCH // 2        # chains per pipeline group (256)
NPIECE = 4            # em DMA pieces (by t-range)
TP = L // NPIECE      # 256 t per piece

CNTC = 2 * T + 4      # count-matrix cols per seq: [i_hi,j] + 4 one-hot ext
AUX_N = T * T + 2 * T # aux: [trans i-major | start | stop]
GSTRIDE = (G + 1) * NCH   # per-jc stride in gbuf


def build_program(debug=False):
    nc = bacc.Bacc()
    em_t = nc.declare_dram_parameter("em", [128 * JCN * BC * L, 1], FP8, isOutput=False)
    em2_t = nc.declare_dram_parameter("em2", [128 * JCN * BC * L, 1], BF16, isOutput=False)
    tags_t = nc.declare_dram_parameter("tags_sc", [BC * L, 1], BF16, isOutput=False)
    cnt_t = nc.declare_dram_parameter("cnt", [128 * BC * CNTC, 1], BF16, isOutput=False)
    aux_t = nc.declare_dram_parameter("aux", [AUX_N, 1], F32, isOutput=False)
    iota_t = nc.declare_dram_parameter("iota", [128, 1], F32, isOutput=False)
    loss_t = nc.declare_dram_parameter("loss", [BC, 1], F32, isOutput=True)
    dbg_t = nc.declare_dram_parameter("dbg", [4096, 1], F32, isOutput=True) if debug else None

    def dram_ap(handle, offset, ap):
        full = handle[:]
        return bass.AP(tensor=full.tensor, offset=offset, ap=ap)

    with tile.TileContext(nc) as tc, ExitStack() as ctx:
        const = ctx.enter_context(tc.tile_pool(name="const", bufs=1))
        stage = ctx.enter_context(tc.tile_pool(name="stage", bufs=2))
        fpA = ctx.enter_context(tc.tile_pool(name="fpA", bufs=3))
        fpB = ctx.enter_context(tc.tile_pool(name="fpB", bufs=3))
        prp = ctx.enter_context(tc.tile_pool(name="prp", bufs=8))
        pp = ctx.enter_context(tc.tile_pool(name="pp", bufs=3, space="PSUM"))
        psp = ctx.enter_context(tc.tile_pool(name="psp", bufs=1, space="PSUM"))

        # ---------------- DMAs --------------------------------------------
        # sync queue: aux + fp8 emissions (scan path, earliest)
        # ACT queue:  tags broadcast + count matrices
        # DVE queue:  bf16 emissions (numerator products)
        iota_sb = const.tile([128, 1], F32, name="iota_sb")
        nc.scalar.dma_start(out=iota_sb, in_=iota_t[:])
        neg_shift = const.tile([128, 1], F32, name="neg_shift")
        nc.vector.memset(neg_shift, -S)

        eraw = [stage.tile([128, T], F32, name=f"eraw{ic}", tag="eraw") for ic in range(JCN)]
        for ic in range(JCN):
            nc.scalar.dma_start(
                out=eraw[ic], in_=dram_ap(aux_t, ic * 128 * T, [[T, 128], [1, T]])
            )
        ssraw = const.tile([128, 2 * JCN], F32, name="ssraw")
        nc.scalar.dma_start(
            out=ssraw[:, 0:JCN], in_=dram_ap(aux_t, T * T, [[1, 128], [128, JCN]])
        )
        nc.scalar.dma_start(
            out=ssraw[:, JCN:2 * JCN],
            in_=dram_ap(aux_t, T * T + T, [[1, 128], [128, JCN]]),
        )

        raw = const.tile([128, JCN, BC, L], FP8, name="raw")
        raw2 = const.tile([128, JCN, BC, L], BF16, name="raw2")
        for p in range(NPIECE):
            dst = bass.AP(
                tensor=raw.tensor,
                offset=raw.offset + p * TP,
                ap=[raw.ap[0], [BC * L, JCN], [L, BC], [1, TP]],
            )
            nc.sync.dma_start(
                out=dst,
                in_=dram_ap(
                    em_t, p * 128 * JCN * BC * TP,
                    [[JCN * BC * TP, 128], [1, JCN * BC * TP]],
                ),
            )
        for p in range(NPIECE):
            dst = bass.AP(
                tensor=raw2.tensor,
                offset=raw2.offset + p * TP,
                ap=[raw2.ap[0], [BC * L, JCN], [L, BC], [1, TP]],
            )
            nc.sync.dma_start(
                out=dst,
                in_=dram_ap(
                    em2_t, p * 128 * JCN * BC * TP,
                    [[JCN * BC * TP, 128], [1, JCN * BC * TP]],
                ),
            )

        tags_bc = const.tile([128, BC * L], BF16, name="tags_bc")
        nc.gpsimd.dma_start(
            out=tags_bc, in_=dram_ap(tags_t, 0, [[0, 128], [1, BC * L]])
        )
        cnt_sb = const.tile([128, BC, CNTC], BF16, name="cnt_sb")
        nc.gpsimd.dma_start(
            out=cnt_sb, in_=dram_ap(cnt_t, 0, [[BC * CNTC, 128], [1, BC * CNTC]])
        )

        # ---------------- weights + start/stop ----------------
        e_tiles = []
        trext = const.tile([128, CNTC], BF16, name="trext")
        for ic in range(JCN):
            ebf = const.tile([128, T], FP8, name=f"ebf{ic}")
            nc.scalar.activation(out=ebf, in_=eraw[ic], func=mybir.ActivationFunctionType.Exp)
            e_tiles.append(ebf)
            nc.vector.tensor_copy(out=trext[:, ic * T:(ic + 1) * T], in_=eraw[ic])
        nc.vector.tensor_copy(out=trext[:, 2 * T:2 * T + 4], in_=ssraw)
        sstart = const.tile([128, JCN], F32, name="sstart")
        nc.scalar.activation(
            out=sstart, in_=ssraw[:, 0:JCN], func=mybir.ActivationFunctionType.Exp
        )
        sstop_bf = const.tile([128, JCN], BF16, name="sstop_bf")
        nc.scalar.activation(
            out=sstop_bf, in_=ssraw[:, JCN:2 * JCN], func=mybir.ActivationFunctionType.Exp
        )
        ones_col = const.tile([128, 1], BF16, name="ones_col")
        nc.vector.memset(ones_col, 1.0)

        # ---------------- g = exp(emit - S), chain-major -------------------
        # gbuf [128, jc, chain, slab]; chain = c*BC + b; slab s>=1 applies
        # g(t): chain 0: t = s; chains c>=1: t = c*K - W + s - 1.
        # chain-major keeps the ACT exp writes slab-contiguous (strided ACT
        # output measured 6.3us/call on hw); the fused drain reads g strided
        # which costs nothing extra (PSUM input already blocks DVE 2x mode).
        gbuf = const.tile([128, JCN, NCH, G + 1], BF16, name="gbuf")

        def emit_exp_chain0(jc):
            out_ap = bass.AP(
                tensor=gbuf.tensor,
                offset=gbuf.offset + jc * GSTRIDE,
                ap=[gbuf.ap[0], [G + 1, BC], [1, G + 1]],
            )
            in_ap = bass.AP(
                tensor=raw.tensor,
                offset=raw.offset + jc * BC * L,
                ap=[raw.ap[0], [L, BC], [1, G + 1]],
            )
            nc.scalar.activation(
                out=out_ap, in_=in_ap, func=mybir.ActivationFunctionType.Exp,
                bias=neg_shift[:],
            )

        def emit_exp(jc, c0, ncnk):
            # chunks c0..c0+ncnk: slabs 1..G  <->  t = c*K - W + s - 1
            out_ap = bass.AP(
                tensor=gbuf.tensor,
                offset=gbuf.offset + jc * GSTRIDE + c0 * BC * (G + 1) + 1,
                ap=[gbuf.ap[0], [BC * (G + 1), ncnk], [G + 1, BC], [1, G]],
            )
            in_ap = bass.AP(
                tensor=raw.tensor,
                offset=raw.offset + jc * BC * L + c0 * KK - W,
                ap=[raw.ap[0], [KK, ncnk], [L, BC], [1, G]],
            )
            nc.scalar.activation(
                out=out_ap, in_=in_ap, func=mybir.ActivationFunctionType.Exp,
                bias=neg_shift[:],
            )

        # A group: chunks 0..31 (em pieces 0,1); B group: chunks 32..63
        for jc in range(JCN):
            emit_exp_chain0(jc)
        for jc in range(JCN):
            emit_exp(jc, 1, 15)
            emit_exp(jc, 16, 16)
        for jc in range(JCN):
            emit_exp(jc, 32, 16)
            emit_exp(jc, 48, 16)

        # ---------------- one-hot masks ----------------
        oh_tiles = [const.tile([128, BC * L], BF16, name=f"oh{jc}") for jc in range(JCN)]
        for jc in range(JCN):
            for hh in range(2):
                lo, hi = hh * (BC * L // 2), (hh + 1) * (BC * L // 2)
                nc.vector.tensor_scalar(
                    out=oh_tiles[jc][:, lo:hi],
                    in0=tags_bc[:, lo:hi],
                    scalar1=float(jc * 128),
                    scalar2=iota_sb[:],
                    op0=mybir.AluOpType.subtract,
                    op1=mybir.AluOpType.is_equal,
                )

        # ---------------- numerator side-jobs (interleaved into the scan) --
        # psE [1, 512]: per-seq [b*64, 64] accumulation region; emissions
        # (2 jc x 2 halves) + count products all column-summed into it.
        psE = psp.tile([1, 8 * 64], F32, name="psE", tag="psE")
        psE2 = psp.tile([1, 8 * 64], F32, name="psE2", tag="psE2")
        seq_mm_count = [0] * BC
        SEQ_MM_TOTAL = JCN * 2 * 8 + 9  # 16 emission-chunk mms x2 + 9 count mms

        def emit_prod_tt(job, eng):
            kind = job[0]
            if kind == "emis":
                _, jc, b, hh = job
                lo = hh * (L // 2)
                tg = "prod" if eng is nc.gpsimd else "prodD"
                pr = prp.tile([128, L // 2], BF16, name="prod", tag=tg)
                eng.tensor_tensor(
                    out=pr,
                    in0=bass.AP(
                        tensor=raw2.tensor,
                        offset=raw2.offset + jc * BC * L + b * L + lo,
                        ap=[raw2.ap[0], [1, L // 2]],
                    ),
                    in1=oh_tiles[jc][:, b * L + lo: b * L + lo + L // 2],
                    op=mybir.AluOpType.mult,
                )
            else:
                _, b = job
                pr = prp.tile([128, CNTC], BF16, name="prodT", tag="prodT")
                eng.tensor_tensor(
                    out=pr, in0=cnt_sb[:, b, :], in1=trext, op=mybir.AluOpType.mult
                )
            return pr

        def mm_entries(pr, b, ps):
            n = (pr.shape[-1] + 63) // 64
            return [(pr, k, b, ps) for k in range(n)]

        def emit_mm(e, start, stop):
            pr, k, b, ps = e
            c0 = k * 64
            c1 = min((k + 1) * 64, pr.shape[-1])
            nc.tensor.matmul(
                out=ps[:, b * 64:b * 64 + (c1 - c0)],
                lhsT=ones_col,
                rhs=pr[:, c0:c1],
                start=start,
                stop=stop,
            )

        def emit_prod_job(job, eng, start, stop, ps=None):
            pr = emit_prod_tt(job, eng)
            ents = mm_entries(pr, job[2] if job[0] == "emis" else job[1], ps)
            for i, e in enumerate(ents):
                emit_mm(e, start and i == 0, stop and i == len(ents) - 1)

        # ONE accumulation group per PSUM region (the scheduler reorders
        # groups freely, so start=False continuation groups get erased by a
        # later-scheduled start=True group).  GPSIMD products (emission
        # half-0 + count) accumulate in psE; DVE products (half-1) in psE2.
        gp_mms = []
        for b in range(BC):
            ents = []
            for jc in range(JCN):
                ents += mm_entries(emit_prod_tt(("emis", jc, b, 0), nc.gpsimd), b, psE)
            ents += mm_entries(emit_prod_tt(("cnt", b), nc.gpsimd), b, psE)
            gp_mms.append(ents)
        gp_flat = []
        for ents in gp_mms:
            for i, e in enumerate(ents):
                gp_flat.append((e, i == 0, i == len(ents) - 1))
        hh1_jobs = [("emis", jc, b, 1) for b in range(BC) for jc in range(JCN)]

        # ---------------- scan init ----------------
        fwA = fpA.tile([128, JCN, GRP], BF16, name="fwA", tag="fwA")
        fwB = fpB.tile([128, JCN, GRP], BF16, name="fwB", tag="fwB")
        nc.vector.memset(fwA, 1.0)
        nc.vector.memset(fwB, 1.0)
        for jc in range(JCN):
            nc.vector.tensor_scalar_mul(
                out=fwA[:, jc, 0:BC],
                in0=bass.AP(tensor=gbuf.tensor, offset=gbuf.offset + jc * GSTRIDE,
                            ap=[gbuf.ap[0], [G + 1, BC]]),
                scalar1=sstart[:, jc:jc + 1],
            )

        psH = psp.tile([1, NCH], F32, name="psH", tag="psH")
        psPsi = psp.tile([1, NCH], F32, name="psPsi", tag="psPsi")
        psB = psp.tile([1, 2 * BC], F32, name="psB", tag="psB")

        def g_drain_view(grp_base, s):
            return bass.AP(
                tensor=gbuf.tensor,
                offset=gbuf.offset + grp_base * (G + 1) + s,
                ap=[gbuf.ap[0], [GSTRIDE, JCN], [G + 1, GRP]],
            )

        def g_drain_view_o(grp_base, o, s):
            return bass.AP(
                tensor=gbuf.tensor,
                offset=gbuf.offset + o * GSTRIDE + grp_base * (G + 1) + s,
                ap=[gbuf.ap[0], [G + 1, GRP]],
            )

        def scan_slab(fwA_t, fwB_t, s):
            pfA = pp.tile([128, JCN, GRP], F32, name="pfA", tag="pf")
            pfB = pp.tile([128, JCN, GRP], F32, name="pfB", tag="pf")
            fwA2 = fpA.tile([128, JCN, GRP], BF16, name="fwA", tag="fwA")
            fwB2 = fpB.tile([128, JCN, GRP], BF16, name="fwB", tag="fwB")
            for (pf, fw_t) in ((pfA, fwA_t), (pfB, fwB_t)):
                for o in range(JCN):
                    for ic in range(JCN):
                        nc.tensor.matmul(
                            out=pf[:, o, :],
                            lhsT=e_tiles[ic][:, o * 128:(o + 1) * 128],
                            rhs=fw_t[:, ic, :],
                            start=(ic == 0),
                            stop=(ic == JCN - 1),
                        )
            for (pf, fw2, gb) in ((pfA, fwA2, 0), (pfB, fwB2, GRP)):
                nc.vector.tensor_tensor(
                    out=fw2, in0=pf,
                    in1=bass.AP(tensor=gbuf.tensor,
                                offset=gbuf.offset + gb * (G + 1) + s,
                                ap=[gbuf.ap[0], [GSTRIDE, JCN], [G + 1, GRP]]),
                    op=mybir.AluOpType.mult,
                )
            return fwA2, fwB2

        def extract_colsums(ps_region, fw_tile, lhsT, c0=0, n=GRP):
            for jc in range(JCN):
                nc.tensor.matmul(
                    out=ps_region,
                    lhsT=lhsT,
                    rhs=fw_tile[:, jc, c0:c0 + n],
                    start=(jc == 0),
                    stop=(jc == JCN - 1),
                )

        for s in range(1, G + 1):
            fwA, fwB = scan_slab(fwA, fwB, s)
            if 6 <= s <= 13:
                for j in range(2):
                    jc_, b_ = hh1_jobs[(s - 6) * 2 + j][1], hh1_jobs[(s - 6) * 2 + j][2]
                    emit_prod_job(("emis", jc_, b_, 1), nc.vector,
                                  start=(jc_ == 0), stop=(jc_ == JCN - 1), ps=psE2)
            lo_m = (s - 1) * len(gp_flat) // 16
            hi_m = s * len(gp_flat) // 16 if s <= 16 else len(gp_flat)
            if s <= 16:
                for (e, st_, sp_) in gp_flat[lo_m:hi_m]:
                    emit_mm(e, st_, sp_)
            if s == W:
                extract_colsums(psH[:, 0:GRP], fwA, ones_col)
                extract_colsums(psH[:, GRP:NCH], fwB, ones_col)
            if s == KK - 1:
                extract_colsums(psB[:, 0:BC], fwA, ones_col, 0, BC)
            if s == G:
                extract_colsums(psPsi[:, 0:GRP], fwA, ones_col)
                extract_colsums(psPsi[:, GRP:NCH], fwB, ones_col)
                for jc in range(JCN):
                    nc.tensor.matmul(
                        out=psB[:, BC:2 * BC],
                        lhsT=sstop_bf[:, jc:jc + 1],
                        rhs=fwB[:, jc, GRP - BC:GRP],
                        start=(jc == 0),
                        stop=(jc == JCN - 1),
                    )

        # ---------------- finalization ----------------
        lnH = const.tile([1, NCH], F32, name="lnH")
        nc.scalar.activation(out=lnH, in_=psH, func=mybir.ActivationFunctionType.Ln)
        lnPsi = const.tile([1, NCH], F32, name="lnPsi")
        nc.scalar.activation(out=lnPsi, in_=psPsi, func=mybir.ActivationFunctionType.Ln)
        lnB = const.tile([1, 2 * BC], F32, name="lnB")
        nc.scalar.activation(out=lnB, in_=psB, func=mybir.ActivationFunctionType.Ln)

        SH = const.tile([1, BC], F32, name="SH")
        nc.vector.tensor_reduce(
            out=SH,
            in_=bass.AP(tensor=lnH.tensor, offset=lnH.offset + BC,
                        ap=[lnH.ap[0], [1, BC], [BC, CCH - 1]]),
            axis=mybir.AxisListType.X, op=mybir.AluOpType.add,
        )
        SA = const.tile([1, BC], F32, name="SA")
        nc.vector.tensor_reduce(
            out=SA,
            in_=bass.AP(tensor=lnPsi.tensor, offset=lnPsi.offset + BC,
                        ap=[lnPsi.ap[0], [1, BC], [BC, CCH - 2]]),
            axis=mybir.AxisListType.X, op=mybir.AluOpType.add,
        )
        numer1 = const.tile([1, BC], F32, name="numer1")
        nc.vector.tensor_reduce(
            out=numer1,
            in_=bass.AP(tensor=psE.tensor, offset=psE.offset,
                        ap=[psE.ap[0], [64, BC], [1, 64]]),
            axis=mybir.AxisListType.X, op=mybir.AluOpType.add,
        )
        numer2 = const.tile([1, BC], F32, name="numer2")
        nc.vector.tensor_reduce(
            out=numer2,
            in_=bass.AP(tensor=psE2.tensor, offset=psE2.offset,
                        ap=[psE2.ap[0], [64, BC], [1, 64]]),
            axis=mybir.AxisListType.X, op=mybir.AluOpType.add,
        )
        numer = const.tile([1, BC], F32, name="numer")
        nc.vector.tensor_tensor(out=numer, in0=numer1, in1=numer2, op=mybir.AluOpType.add)
        # loss = (numer - L*S) + (SH - SA - lnP0 - lnStop)
        u1 = const.tile([1, BC], F32, name="u1")
        nc.vector.tensor_tensor(out=u1, in0=SH, in1=SA, op=mybir.AluOpType.subtract)
        u2 = const.tile([1, BC], F32, name="u2")
        nc.vector.tensor_tensor(out=u2, in0=u1, in1=lnB[:, 0:BC], op=mybir.AluOpType.subtract)
        u3 = const.tile([1, BC], F32, name="u3")
        nc.vector.tensor_tensor(out=u3, in0=u2, in1=lnB[:, BC:2 * BC], op=mybir.AluOpType.subtract)
        loss_sb = const.tile([1, BC], F32, name="loss_sb")
        nc.vector.scalar_tensor_tensor(
            out=loss_sb,
            in0=numer,
            scalar=float(L * S),
            in1=u3,
            op0=mybir.AluOpType.subtract,
            op1=mybir.AluOpType.add,
        )
        nc.sync.dma_start(out=dram_ap(loss_t, 0, [[1, 1], [1, BC]]), in_=loss_sb)
        if debug:
            psOH = psp.tile([1, 512], F32, name="psOH", tag="psOH")
            for jc in range(JCN):
                nc.tensor.matmul(out=psOH, lhsT=ones_col, rhs=oh_tiles[jc][:, 0:512],
                                 start=(jc == 0), stop=(jc == JCN - 1))
            psOH_sb = const.tile([1, 512], F32, name="psOH_sb")
            nc.vector.tensor_copy(out=psOH_sb, in_=psOH)
            nc.sync.dma_start(out=dram_ap(dbg_t, 2 * NCH + 3 * BC + 512, [[1, 1], [1, 512]]), in_=psOH_sb)
            r2d = const.tile([1, 512], F32, name="r2d")
            nc.vector.tensor_copy(
                out=r2d,
                in_=bass.AP(tensor=raw2.tensor, offset=raw2.offset,
                            ap=[[raw2.ap[0][0], 1], [1, 512]]),
            )
            nc.sync.dma_start(out=dram_ap(dbg_t, 2 * NCH + 3 * BC + 1024, [[1, 1], [1, 512]]), in_=r2d)
            psE_sb = const.tile([1, 8 * 64], F32, name="psE_sb")
            nc.vector.tensor_copy(out=psE_sb, in_=psE)
            nc.sync.dma_start(out=dram_ap(dbg_t, 2 * NCH + 3 * BC, [[1, 1], [1, 8 * 64]]), in_=psE_sb)
            nc.sync.dma_start(out=dram_ap(dbg_t, 0, [[1, 1], [1, NCH]]), in_=lnH)
            nc.sync.dma_start(out=dram_ap(dbg_t, NCH, [[1, 1], [1, NCH]]), in_=lnPsi)
            nc.sync.dma_start(out=dram_ap(dbg_t, 2 * NCH, [[1, 1], [1, 2 * BC]]), in_=lnB)
            nc.sync.dma_start(out=dram_ap(dbg_t, 2 * NCH + 2 * BC, [[1, 1], [1, BC]]), in_=numer)

    nc.finalize()
    return nc


def host_inputs(inputs, tags, length=L):
    """Per-core relaid inputs (host-side sharding / layout / int bookkeeping)."""
    inputs = np.asarray(inputs, dtype=np.float32)
    tags = np.asarray(tags).astype(np.int64)
    fp8 = mybir.dt.np(FP8)
    bf16 = mybir.dt.np(BF16)

    in_maps = []
    for cc in range(NCORES):
        bsl = slice(cc * BC, (cc + 1) * BC)
        x = inputs[bsl].reshape(BC, length, JCN, 128)
        # em[p, part, jc, seq, t'] with t = p*TP + t'
        em = np.ascontiguousarray(
            x.reshape(BC, NPIECE, TP, JCN, 128).transpose(1, 4, 3, 0, 2)
        )
        tg = tags[bsl].astype(bf16).reshape(-1)
        cnt = np.zeros((128, BC, CNTC), np.float32)
        tgs = tags[bsl]
        for b in range(BC):
            c2 = np.zeros((128, 2, T), np.float32)
            np.add.at(c2, (tgs[b, :-1] % 128, tgs[b, :-1] // 128, tgs[b, 1:]), 1.0)
            cnt[:, b, :2 * T] = c2.reshape(128, 2 * T)
            cnt[tgs[b, 0] % 128, b, 2 * T + tgs[b, 0] // 128] = 1.0
            cnt[tgs[b, -1] % 128, b, 2 * T + 2 + tgs[b, -1] // 128] = 1.0
        in_maps.append(dict(
            em=em.astype(fp8).reshape(-1, 1),
            em2=em.astype(bf16).reshape(-1, 1),
            tags_sc=tg.reshape(-1, 1),
            cnt=cnt.astype(bf16).reshape(-1, 1),
        ))
    return in_maps


def host_shared(transitions, start_transitions, stop_transitions):
    aux = np.zeros((AUX_N, 1), np.float32)
    aux[:T * T, 0] = np.asarray(transitions, np.float32).reshape(-1)  # i-major
    aux[T * T:T * T + T, 0] = np.asarray(start_transitions, np.float32)
    aux[T * T + T:, 0] = np.asarray(stop_transitions, np.float32)
    iota = np.arange(128, dtype=np.float32).reshape(128, 1)
    return dict(aux=aux, iota=iota)


def kernel(inputs, tags, mask, transitions, start_transitions, stop_transitions):
    del mask  # all-ones per the problem spec
    in_maps = host_inputs(inputs, tags)
    shared = host_shared(transitions, start_transitions, stop_transitions)
    for m in in_maps:
        m.update(shared)

    nc = build_program()
    res = run_bass_kernel_spmd(nc, in_maps, core_ids=list(range(NCORES)))
    out = np.concatenate([r["loss"].reshape(BC) for r in res.results])
    return out.astype(np.float32)


if __name__ == "__main__":
    rng = np.random.default_rng(0)
    inputs = rng.standard_normal((B, L, T), dtype=np.float32)
    tags = rng.integers(0, T, size=(B, L))
    trans = rng.standard_normal((T, T)).astype(np.float32)
    start = rng.standard_normal(T).astype(np.float32)
    stop = rng.standard_normal(T).astype(np.float32)
    out = kernel(inputs, tags, np.ones((B, L), bool), trans, start, stop)
    print(out)
